# revision 7
# baseline (speedup 1.0000x reference)
"""Trainium2 Bass kernel for the pointer-network decoder (nn_Decoder).

Data-parallel over batch: B=512 split as 64 samples on each of 8 NeuronCores.
Inside each core, a 128-step sequential scan:
  LSTM cell -> glimpse attention -> pointer attention -> gumbel-argmax sample
  -> embedding gather -> mask update.

Sampling replicates jax.random.categorical(key, lp) == argmax(lp + gumbel)
with the gumbel noise precomputed on the host from the fixed keys.

Layout conventions on device (per core):
  - "T" suffix = transposed [feature-on-partitions, batch-on-free], e.g.
    hxT is [H=128, b=64].
  - big attention tensors are [h=128 partitions, (b, s) = 64*128 free].
  - e_g_SbH is [s=128 partitions, (b, h) free] for the per-sample glimpse
    matvec (contraction over s).
"""
import sys
import numpy as np

sys.path.insert(0, "/opt/trn_rl_repo")

import concourse.bass as bass
import concourse.mybir as mybir
from concourse.bacc import Bacc
from concourse.tile import TileContext
from concourse.bass_utils import run_bass_kernel_spmd

F32 = mybir.dt.float32
I32 = mybir.dt.int32
AF = mybir.ActivationFunctionType
OP = mybir.AluOpType
AX = mybir.AxisListType

NCORES = 8
B_FULL, S, E, H = 512, 128, 128, 128
B = B_FULL // NCORES  # 64 per core
T_STEPS = S
NEG = -1e30
C_SCALE = 10.0
NCHUNK = 4              # b-chunks for the big add/tanh pipeline
BC = B // NCHUNK        # 16 samples per chunk

_cache = {}


def _blob_layout():
    """Column layout of the per-core constant/input blob [128, NBLOB]."""
    cols = {}
    c = 0
    for name, w in [("xT0", B), ("hxT", B), ("cxT", B),
                    ("W_in", 4 * H), ("W_out", 4 * H),
                    ("bias_half", 4), ("bias_full", 4),
                    ("Wq_g", H), ("Wq_p", H), ("Wr_g", H), ("Wr_p", H),
                    ("v_g", 1), ("v_p", 1),
                    ("br_g", 1), ("bq_g", 1), ("brq_p", 1),
                    ("iota_s", S), ("bcol", 1), ("ident", 128)]:
        cols[name] = (c, c + w)
        c += w
    return cols, c


def _build(n_steps, repeats=1):
    cols, nblob = _blob_layout()
    nc = Bacc()

    blob_d = nc.declare_dram_parameter("blob", [128, nblob], F32, isOutput=False)
    enc_d = nc.declare_dram_parameter("enc", [H, B * S], F32, isOutput=False)
    emb_d = nc.declare_dram_parameter("emb", [S * B, E], F32, isOutput=False)
    gum_d = nc.declare_dram_parameter("gum", [n_steps, B, S], F32, isOutput=False)
    probs_d = nc.declare_dram_parameter("probs", [n_steps, B, S], F32, isOutput=True)
    sels_d = nc.declare_dram_parameter("sels", [n_steps, B, 1], I32, isOutput=True)
    hyT_d = nc.declare_dram_parameter("hyT", [H, B], F32, isOutput=True)
    cyT_d = nc.declare_dram_parameter("cyT", [H, B], F32, isOutput=True)

    with TileContext(nc) as tc:
        with tc.tile_pool(name="state", bufs=1) as st, \
             tc.tile_pool(name="big", bufs=1) as bigp, \
             tc.tile_pool(name="work", bufs=2) as wk:

            blob = st.tile([128, nblob], F32)
            nc.sync.dma_start(blob[:], blob_d[:])

            def cpart(name, rows=128):
                c0, c1 = cols[name]
                return blob[:rows, c0:c1]

            xT = st.tile([H, B], F32)
            hxT = st.tile([H, B], F32)
            cxT = st.tile([H, B], F32)
            mask = st.tile([B, S], F32)   # holds mask * (-1e30)

            ident = cpart("ident")
            v_g = cpart("v_g")
            v_p = cpart("v_p")
            iota_s = cpart("iota_s", rows=B)
            bcol = cpart("bcol", rows=B)

            # ---------------- precompute e-tensors ----------------
            e_g = bigp.tile([128, B * S], F32)    # becomes e_g + bq_g (tanh input)
            e_p = bigp.tile([128, B * S], F32)    # e_p + br_p + bq_p
            e_g_sbh = bigp.tile([128, B * H], F32)  # raw e_g in [s, (b,h)] layout

            with tc.tile_pool(name="encp", bufs=1) as encp, \
                 tc.tile_pool(name="psp", bufs=2, space="PSUM") as psp:
                enc_T = encp.tile([128, B * S], F32)
                nc.sync.dma_start(enc_T[:], enc_d[:])

                for c in range(16):
                    sl = slice(c * 512, (c + 1) * 512)
                    pe_g = psp.tile([128, 512], F32, tag="pre_e")
                    nc.tensor.matmul(pe_g[:], cpart("Wr_g"), enc_T[:, sl],
                                     start=True, stop=True)
                    nc.scalar.activation(e_g[:, sl], pe_g[:], AF.Identity,
                                         bias=cpart("br_g")[:, :1], scale=1.0)
                    pe_p = psp.tile([128, 512], F32, tag="pre_e")
                    nc.tensor.matmul(pe_p[:], cpart("Wr_p"), enc_T[:, sl],
                                     start=True, stop=True)
                    nc.scalar.activation(e_p[:, sl], pe_p[:], AF.Identity,
                                         bias=cpart("brq_p")[:, :1], scale=1.0)

                # transpose raw e_g (with br_g, before bq_g fold) per sample
                for b in range(B):
                    sl = slice(b * 128, (b + 1) * 128)
                    pt = psp.tile([128, 128], F32, tag="pre_tr")
                    nc.tensor.transpose(pt[:], e_g[:, sl], ident)
                    nc.scalar.copy(e_g_sbh[:, sl], pt[:])

                # fold bq_g into e_g in place -> tanh input version
                nc.scalar.activation(e_g[:], e_g[:], AF.Identity,
                                     bias=cpart("bq_g")[:, :1], scale=1.0)

            ps_cm = tc.tile_pool(name="ps", bufs=1, space="PSUM")
            ps = ps_cm.__enter__()

            # ---------------- the scan ----------------
            rep_cm = tc.For_i(0, repeats) if repeats != 1 else None
            if rep_cm is not None:
                rep_cm.__enter__()
            nc.vector.tensor_copy(xT[:], cpart("xT0"))
            nc.vector.tensor_copy(hxT[:], cpart("hxT"))
            nc.vector.tensor_copy(cxT[:], cpart("cxT"))
            nc.vector.memset(mask[:], 0.0)
            with tc.For_i(0, n_steps) as t:
                gum = wk.tile([B, S], F32, tag="gum")
                nc.sync.dma_start(
                    gum[:],
                    gum_d[bass.ds(t, 1), :, :].rearrange("o b s -> (o b) s"))

                # LSTM gates: [4H, B] in psum as 4 column-blocks of [128, 64]
                pg = ps.tile([128, 4 * B], F32, tag="gates")
                for j in range(4):
                    osl = slice(j * B, (j + 1) * B)
                    wsl = slice(j * 128, (j + 1) * 128)
                    nc.tensor.matmul(pg[:, osl], cpart("W_in")[:, wsl], xT[:],
                                     start=True, stop=False)
                    nc.tensor.matmul(pg[:, osl], cpart("W_out")[:, wsl], hxT[:],
                                     start=False, stop=True)
                # i, f, o via tanh half-identity; g via plain tanh
                ti = wk.tile([H, B], F32, tag="ti")
                tf = wk.tile([H, B], F32, tag="tf")
                tg = wk.tile([H, B], F32, tag="tg")
                to = wk.tile([H, B], F32, tag="to")
                bh = cpart("bias_half")
                bf = cpart("bias_full")
                nc.scalar.activation(ti[:], pg[:, 0:B], AF.Tanh,
                                     bias=bh[:, 0:1], scale=0.5)
                nc.scalar.activation(tf[:], pg[:, B:2 * B], AF.Tanh,
                                     bias=bh[:, 1:2], scale=0.5)
                nc.scalar.activation(tg[:], pg[:, 2 * B:3 * B], AF.Tanh,
                                     bias=bf[:, 2:3], scale=1.0)
                nc.scalar.activation(to[:], pg[:, 3 * B:4 * B], AF.Tanh,
                                     bias=bh[:, 3:4], scale=0.5)
                si = wk.tile([H, B], F32, tag="si")
                sf = wk.tile([H, B], F32, tag="sf")
                so = wk.tile([H, B], F32, tag="so")
                nc.vector.tensor_scalar(si[:], ti[:], 0.5, 0.5, OP.mult, OP.add)
                nc.vector.tensor_scalar(sf[:], tf[:], 0.5, 0.5, OP.mult, OP.add)
                nc.vector.tensor_scalar(so[:], to[:], 0.5, 0.5, OP.mult, OP.add)
                c1 = wk.tile([H, B], F32, tag="c1")
                c2 = wk.tile([H, B], F32, tag="c2")
                nc.vector.tensor_tensor(c1[:], sf[:], cxT[:], OP.mult)
                nc.vector.tensor_tensor(c2[:], si[:], tg[:], OP.mult)
                nc.vector.tensor_tensor(cxT[:], c1[:], c2[:], OP.add)  # cy
                tcy = wk.tile([H, B], F32, tag="tcy")
                nc.scalar.activation(tcy[:], cxT[:], AF.Tanh)
                nc.vector.tensor_tensor(hxT[:], so[:], tcy[:], OP.mult)  # hy

                def attention(e_tile, wq, v_col, psum_u_tag):
                    """q = wq^T @ hxT-like rhs; u[b,s] = sum_h v tanh(q + e).
                    Returns uT sbuf tile [B, S]."""
                    pq = ps.tile([H, B], F32, tag="pq")
                    nc.tensor.matmul(pq[:], wq, attention.rhs,
                                     start=True, stop=True)
                    pu = ps.tile([S, B], F32, tag="pu")
                    for c in range(NCHUNK):
                        csl = slice(c * BC * S, (c + 1) * BC * S)
                        tin = wk.tile([128, BC * S], F32, tag="tin")
                        qb = pq[:, c * BC:(c + 1) * BC].unsqueeze(2) \
                            .to_broadcast([128, BC, S])
                        nc.vector.tensor_tensor(
                            tin[:].rearrange("p (b s) -> p b s", b=BC),
                            e_tile[:, csl].rearrange("p (b s) -> p b s", b=BC),
                            qb, OP.add)
                        tt = wk.tile([128, BC * S], F32, tag="tt")
                        nc.scalar.activation(tt[:], tin[:], AF.Tanh)
                        for bl in range(BC):
                            b = c * BC + bl
                            nc.tensor.matmul(pu[:, b:b + 1],
                                             tt[:, bl * S:(bl + 1) * S],
                                             v_col, start=True, stop=True)
                    u_sb = wk.tile([S, B], F32, tag="u_sb")
                    nc.scalar.copy(u_sb[:], pu[:])
                    put = ps.tile([B, S], F32, tag="put")
                    nc.tensor.transpose(put[:], u_sb[:], ident)
                    uT = wk.tile([B, S], F32, tag="uT")
                    nc.scalar.copy(uT[:], put[:])
                    return uT

                # ---- glimpse ----
                attention.rhs = hxT[:]
                ugT = attention(e_g, cpart("Wq_g"), v_g, "ug")
                lg = wk.tile([B, S], F32, tag="lg")
                nc.vector.tensor_tensor(lg[:], ugT[:], mask[:], OP.add)
                mxg = wk.tile([B, 1], F32, tag="mxg")
                nc.vector.tensor_reduce(mxg[:, :1], lg[:], axis=AX.X, op=OP.max)
                nmxg = wk.tile([B, 1], F32, tag="nmxg")
                nc.vector.tensor_scalar_mul(nmxg[:], mxg[:], -1.0)
                expg = wk.tile([B, S], F32, tag="expg")
                sumg = wk.tile([B, 1], F32, tag="sumg")
                nc.scalar.activation(expg[:], lg[:], AF.Exp,
                                     bias=nmxg[:, :1], scale=1.0,
                                     accum_out=sumg[:, :1])
                recg = wk.tile([B, 1], F32, tag="recg")
                nc.vector.reciprocal(recg[:], sumg[:, :1])
                wg = wk.tile([B, S], F32, tag="wg")
                nc.vector.tensor_scalar_mul(wg[:], expg[:], recg[:, :1])
                pwT = ps.tile([S, B], F32, tag="pwT")
                nc.tensor.transpose(pwT[:], wg[:], ident[:B, :B])
                wT = wk.tile([S, B], F32, tag="wT")
                nc.scalar.copy(wT[:], pwT[:])
                pgl = ps.tile([H, B], F32, tag="pgl")
                for b in range(B):
                    nc.tensor.matmul(pgl[:, b:b + 1],
                                     e_g_sbh[:, b * H:(b + 1) * H],
                                     wT[:, b:b + 1], start=True, stop=True)
                gl = wk.tile([H, B], F32, tag="gl")
                nc.scalar.copy(gl[:], pgl[:])

                # ---- pointer ----
                attention.rhs = gl[:]
                upT = attention(e_p, cpart("Wq_p"), v_p, "up")
                tup = wk.tile([B, S], F32, tag="tup")
                nc.scalar.activation(tup[:], upT[:], AF.Tanh)
                lp = wk.tile([B, S], F32, tag="lp")
                nc.vector.scalar_tensor_tensor(lp[:], tup[:], C_SCALE, mask[:],
                                               OP.mult, OP.add)
                mxp = wk.tile([B, 1], F32, tag="mxp")
                nc.vector.tensor_reduce(mxp[:, :1], lp[:], axis=AX.X, op=OP.max)
                nmxp = wk.tile([B, 1], F32, tag="nmxp")
                nc.vector.tensor_scalar_mul(nmxp[:], mxp[:], -1.0)
                expp = wk.tile([B, S], F32, tag="expp")
                sump = wk.tile([B, 1], F32, tag="sump")
                nc.scalar.activation(expp[:], lp[:], AF.Exp,
                                     bias=nmxp[:, :1], scale=1.0,
                                     accum_out=sump[:, :1])
                recp = wk.tile([B, 1], F32, tag="recp")
                nc.vector.reciprocal(recp[:], sump[:, :1])
                probs = wk.tile([B, S], F32, tag="probs")
                nc.vector.tensor_scalar_mul(probs[:], expp[:], recp[:, :1])
                nc.sync.dma_start(
                    probs_d[bass.ds(t, 1), :, :].rearrange("o b s -> (o b) s"),
                    probs[:])

                # ---- sample: argmax(lp + gumbel) ----
                z = wk.tile([B, S], F32, tag="z")
                nc.vector.tensor_tensor(z[:], lp[:], gum[:], OP.add)
                mxz = wk.tile([B, 1], F32, tag="mxz")
                nc.vector.tensor_reduce(mxz[:, :1], z[:], axis=AX.X, op=OP.max)
                oh = wk.tile([B, S], F32, tag="oh")
                nc.vector.tensor_scalar(oh[:], z[:], mxz[:, :1], None,
                                        OP.is_equal)
                nc.vector.scalar_tensor_tensor(mask[:], oh[:], NEG, mask[:],
                                               OP.mult, OP.add)
                ohi = wk.tile([B, S], F32, tag="ohi")
                self_f = wk.tile([B, 1], F32, tag="self_f")
                nc.vector.tensor_tensor(ohi[:], oh[:], iota_s, OP.mult)
                nc.vector.tensor_reduce(self_f[:, :1], ohi[:], axis=AX.X,
                                        op=OP.add)
                sel_i = wk.tile([B, 1], I32, tag="sel_i")
                nc.vector.tensor_copy(sel_i[:], self_f[:])
                nc.sync.dma_start(
                    sels_d[bass.ds(t, 1), :, :].rearrange("o b one -> (o b) one"),
                    sel_i[:])
                rowf = wk.tile([B, 1], F32, tag="rowf")
                nc.vector.scalar_tensor_tensor(rowf[:], self_f[:], float(B),
                                               bcol[:, :1], OP.mult, OP.add)
                rowi = wk.tile([B, 1], I32, tag="rowi")
                nc.vector.tensor_copy(rowi[:], rowf[:])
                xrows = wk.tile([B, E], F32, tag="xrows")
                nc.gpsimd.indirect_dma_start(
                    out=xrows[:], out_offset=None, in_=emb_d[:],
                    in_offset=bass.IndirectOffsetOnAxis(ap=rowi[:, :1], axis=0))
                pxT = ps.tile([E, B], F32, tag="pxT")
                nc.tensor.transpose(pxT[:], xrows[:], ident[:B, :B])
                nc.scalar.copy(xT[:], pxT[:])

            if rep_cm is not None:
                rep_cm.__exit__(None, None, None)
            nc.sync.dma_start(hyT_d[:], hxT[:])
            nc.sync.dma_start(cyT_d[:], cxT[:])
            ps_cm.__exit__(None, None, None)

    nc.finalize()
    return nc, cols, nblob


def _host_prep(inputs, n_steps):
    """Build per-core input maps from the full inputs."""
    import jax

    d = {k: np.ascontiguousarray(np.asarray(v)) for k, v in inputs.items()}
    cols, nblob = _blob_layout()

    with jax.default_device(jax.devices("cpu")[0]):
        keys = jax.random.split(jax.random.key(42), S)
        # must be drawn per-key to stay bit-identical with
        # jax.random.categorical(key, lp) == argmax(lp + gumbel(key))
        gum_fn = jax.jit(lambda k: jax.random.gumbel(k, (B_FULL, S), "float32"))
        gum_full = np.stack([np.asarray(gum_fn(k)) for k in keys])

    bias_lstm = (d["b_in"] + d["b_out"]).astype(np.float32)      # [4H]
    bias_half4 = 0.5 * bias_lstm.reshape(4, 128).T               # [128, 4]
    bias_full4 = bias_lstm.reshape(4, 128).T                     # [128, 4]
    iota = np.broadcast_to(np.arange(S, dtype=np.float32), (B, S))
    bcol = np.arange(B, dtype=np.float32)[:, None]
    ident = np.eye(128, dtype=np.float32)

    in_maps = []
    for c in range(NCORES):
        bs = slice(c * B, (c + 1) * B)
        blob = np.zeros((128, nblob), np.float32)

        def put(name, arr, rows=128):
            c0, c1 = cols[name]
            a = np.asarray(arr, np.float32)
            assert a.shape == (rows, c1 - c0), (name, a.shape)
            blob[:rows, c0:c1] = a

        put("xT0", d["decoder_input"][bs].T)
        put("hxT", d["hx"][bs].T)
        put("cxT", d["cx"][bs].T)
        put("W_in", d["W_in"])
        put("W_out", d["W_out"])
        put("bias_half", bias_half4)
        put("bias_full", bias_full4)
        put("Wq_g", d["Wq_g"])
        put("Wq_p", d["Wq_p"])
        put("Wr_g", d["Wr_g"])
        put("Wr_p", d["Wr_p"])
        put("v_g", d["v_g"][:, None])
        put("v_p", d["v_p"][:, None])
        put("br_g", d["br_g"][:, None])
        put("bq_g", d["bq_g"][:, None])
        put("brq_p", (d["br_p"] + d["bq_p"])[:, None])
        put("iota_s", iota, rows=B)
        put("bcol", bcol, rows=B)
        put("ident", ident)

        enc = np.ascontiguousarray(
            d["encoder_outputs"][:, bs, :].transpose(2, 1, 0)).reshape(H, B * S)
        emb = np.ascontiguousarray(d["embbed_inputs"][:, bs, :]).reshape(S * B, E)
        gum = np.ascontiguousarray(gum_full[:n_steps, bs, :])

        in_maps.append({"blob": blob, "enc": enc, "emb": emb, "gum": gum})
    return in_maps


def run_sharded(inputs, n_steps=T_STEPS, trace=False, repeats=1):
    key = ("nc", n_steps, repeats)
    if key not in _cache:
        _cache[key] = _build(n_steps, repeats)
    nc, cols, nblob = _cache[key]
    in_maps = _host_prep(inputs, n_steps)
    res = run_bass_kernel_spmd(nc, in_maps, list(range(NCORES)), trace=trace)

    probs = np.zeros((n_steps, B_FULL, S), np.float32)
    sels = np.zeros((n_steps, B_FULL), np.int32)
    hy = np.zeros((B_FULL, H), np.float32)
    cy = np.zeros((B_FULL, H), np.float32)
    for c in range(NCORES):
        bs = slice(c * B, (c + 1) * B)
        r = res.results[c]
        probs[:, bs, :] = r["probs"]
        sels[:, bs] = r["sels"][:, :, 0]
        hy[bs] = r["hyT"].T
        cy[bs] = r["cyT"].T
    return (probs, sels, hy, cy), res


def kernel(**inputs):
    out, _ = run_sharded(inputs)
    return out
